# revision 39
# baseline (speedup 1.0000x reference)
"""Trainium2 Bass kernel for nn_AbstractAttention (dense transformer attention
with RoPE, B=2 S=2048 D=4096 H=32), tensor-parallel over heads on 8 cores.

Single-pass bf16 design; all matmuls in "flipped" orientations (no PE
transposes):

  q^T/k^T [dl, t]  = (wT chunk)^T @ x^T      (weight stationary)
  v [t, e]         = (x^T chunk)^T @ wvT     (x stationary)
  s^T [k, q]       = (k^T chunk)^T @ q^T     (k stationary)
  ctx^T [hd, q]    = (v chunk)^T @ p^T       (v stationary)  <- accumulates
  out^T [e, t]     = (woT chunk)^T @ ctx^T   (wo stationary)

v3 changes over the 1.27ms baseline (-> ~1.12ms, 92% PE MFU):
  - causal trimming: score/exp/PV/den matmuls only stream each key-chunk's
    valid query window (exact; saves 15% of attention flops/exp columns).
  - the causal triangle is applied multiplicatively (0/1 bf16 tile) to the
    exp output on the GpSimd/Pool engine, so the DVE only runs softmax
    tails and the ACT->PV chain has no DVE hop.
  - ctx stays resident in SBUF (no DRAM round-trip before out-proj).
  - attention loops (b, qt, h) so ctx for token-group tg completes in tg
    order; block-level software pipelining with 4 score banks in flight
    and per-iteration tails deferred into the next iteration.
  - rope tails (pair-swap matmul + mixes) deferred one token-group so the
    PE never waits on the ACT copies.
  - phase 3: two-bank PSUM pairs, copies alternate ACT/DVE, out DMAs
    round-robin across 3 queues.
  - weights DMA'd in sixteenths, x tiles in halves (faster first matmul).

Measured per-phase (span/idle us): v 236/8, qk 468/6, attention 194/5,
out-proj 204/1. Attention is exp-throughput-bound on ACT; projections are
at the PE bf16 streaming limit, so the remaining headroom is ~2%.
(Tried and rejected: fp8 DoubleRow (e4m3 noise blows the 2e-2 gate),
Pool-engine denominator accumulation (tensor ops ~1.1us/tile, too slow),
paired 1024-wide exps (needs 6 score banks; PSUM only has 8 total and
ctx/den double-buffering needs 4), DVE denominator adds (attention is
ACT-bound, so removing PE work gains nothing and perturbs the pipeline).)

Softmax skips max-subtraction (scores are O(1)); the denominator is a
ones-row matmul partition-reduce riding the PE; normalization happens on
the small context via reciprocal on DVE.

Host: shards weights by head, preps bf16 transposed layouts, sums the 8
partial out^T tensors in fp32 and transposes back.
"""

import math

import numpy as np

import concourse.bacc as bacc
import concourse.mybir as mybir
from concourse.bass_utils import run_bass_kernel_spmd
from concourse.tile import TileContext

P = 128
F32 = mybir.dt.float32
BF16 = mybir.dt.bfloat16
ALU = mybir.AluOpType
ACTF = mybir.ActivationFunctionType

# set by test.py for profiling; grading path leaves these alone
TRACE = False
TRACE_DIR = None
LAST_RESULT = [None]


def _analyze_blocks(maskT, S, QT):
    """maskT: [S, S] (k, q). Per (qt, kc) block: None if fully masked, else
    (q0, tri) with q0 the first valid query column (suffix window) and tri
    True if the leading [128, 128] of the window needs the causal triangle.
    Requires a pure 0/-inf mask whose masked region per block is exactly a
    suffix-window + leading lower-triangle (true for causal)."""
    nqt, nkc = S // QT, S // P
    blocks = [[None] * nkc for _ in range(nqt)]
    tri = np.triu(np.ones((P, P), dtype=np.float32))
    neg = np.isneginf(maskT) | (maskT < -1e30)
    assert np.all((maskT == 0.0) | neg), "mask must be 0/-inf"
    for qt in range(nqt):
        for kc in range(nkc):
            sub = neg[kc * P : (kc + 1) * P, qt * QT : (qt + 1) * QT]
            col_masked = sub.all(axis=0)
            if col_masked.all():
                continue
            q0 = int(np.argmax(~col_masked))
            assert not col_masked[q0:].any() or col_masked[q0:].sum() == 0
            win = sub[:, q0:]
            if not win.any():
                blocks[qt][kc] = (q0, False)
            else:
                assert win.shape[1] >= P, "triangle wider than window"
                got = (~win[:, :P]).astype(np.float32)
                assert np.array_equal(got, tri), "non-causal block pattern"
                assert not win[:, P:].any()
                blocks[qt][kc] = (q0, True)
    return blocks


def _build(B, S, D, HL, blocks):
    """Per-core Bass program. HL local heads, DL=HL*128 local dims."""
    DL = HL * P
    T = B * S
    KD = D // P        # 32 K-chunks of the model dim
    NTG = T // 512     # 8 token groups of 512
    NKC = S // P       # 16 key chunks per batch
    NQT = S // 512     # 4 query tiles per batch
    inv_sqrt_d = 1.0 / math.sqrt(P)

    nc = bacc.Bacc(None, target_bir_lowering=False)

    xT = nc.declare_dram_parameter("xT", [D, T], BF16, isOutput=False)
    wqT = nc.declare_dram_parameter("wqT", [D, DL], BF16, isOutput=False)
    wkT = nc.declare_dram_parameter("wkT", [D, DL], BF16, isOutput=False)
    wvT = nc.declare_dram_parameter("wvT", [D, DL], BF16, isOutput=False)
    woT = nc.declare_dram_parameter("woT", [DL, D], BF16, isOutput=False)
    cosE = nc.declare_dram_parameter("cosE", [P, S], BF16, isOutput=False)
    sinS = nc.declare_dram_parameter("sinS", [P, S], BF16, isOutput=False)
    pswap = nc.declare_dram_parameter("pswap", [P, P], BF16, isOutput=False)
    trid = nc.declare_dram_parameter("tri", [P, P], BF16, isOutput=False)
    outT = nc.declare_dram_parameter("outT", [D, T], BF16, isOutput=True)

    ts = lambda i, s: slice(i * s, (i + 1) * s)

    xT_r = xT.ap().rearrange("(o p) t -> p o t", p=P)
    woT_r = woT.ap().rearrange("(o p) e -> p o e", p=P)
    outT_r = outT.ap().rearrange("(o p) t -> p o t", p=P)

    with TileContext(nc) as tc:
        with (
            tc.tile_pool(name="res", bufs=1) as res,
            tc.tile_pool(name="consts", bufs=1) as consts,
        ):
            # resident q^T/k^T (roped, bf16) and v
            qres = res.tile([P, HL, T], BF16)
            kres = res.tile([P, HL, T], BF16)
            vres = res.tile([P, T // P, DL], BF16)  # [k-in-chunk, t-chunk, (h,hd)]

            ones_mat = consts.tile([P, P], BF16)
            nc.vector.memset(ones_mat, 1.0)

            # ============ Phase 1: projections, order v -> q -> k ============
            with (
                tc.tile_pool(name="p1c", bufs=1) as p1c,
                tc.tile_pool(name="wpool", bufs=2) as wpool,
                tc.tile_pool(name="xpool", bufs=7) as xpool,
                tc.tile_pool(name="rp", bufs=1) as rp,
                tc.tile_pool(name="psacc", bufs=6, space="PSUM") as psacc,
                tc.tile_pool(name="pssw", bufs=2, space="PSUM") as pssw,
            ):
                cos_sb = p1c.tile([P, S], BF16)
                nc.gpsimd.dma_start(cos_sb, cosE.ap())
                sin_sb = p1c.tile([P, S], BF16)
                nc.gpsimd.dma_start(sin_sb, sinS.ap())
                psw_sb = p1c.tile([P, P], BF16)
                nc.gpsimd.dma_start(psw_sb, pswap.ap())

                def load_w(src, sync_from=16):
                    # scalar-queue issue: keeps x tiles (sync queue) unblocked
                    # sixteenth-granularity so the first matmuls start early;
                    # sixteenths >= sync_from ride the sync queue so the
                    # first token-group is not scalar-bandwidth starved
                    six = []
                    r = src.ap().rearrange("(o p) n -> p o n", p=P)
                    for ke in range(16):
                        t = wpool.tile([P, 2, DL], BF16, tag=f"w{ke}")
                        q = nc.scalar if ke < sync_from else nc.sync
                        if ke == 0:
                            # split the very first tile so matmul 0 can
                            # start after a quarter-MB
                            q.dma_start(t[:, 0, :], r[:, ke * 2, :])
                            q.dma_start(t[:, 1, :], r[:, ke * 2 + 1, :])
                        else:
                            q.dma_start(t, r[:, ts(ke, 2), :])
                        six.append(t)
                    return six

                def load_x(kq, tg, first=False):
                    # two half tiles so the first matmuls start earlier
                    xa = xpool.tile([P, 4, 512], BF16, tag="xt", name=f"xa{kq}{tg}")
                    if first:
                        # quarter-granularity for the very first load so
                        # matmul 0 starts as early as possible
                        nc.sync.dma_start(
                            xa[:, 0:2, :], xT_r[:, kq * 8 : kq * 8 + 2, ts(tg, 512)]
                        )
                        nc.sync.dma_start(
                            xa[:, 2:4, :],
                            xT_r[:, kq * 8 + 2 : kq * 8 + 4, ts(tg, 512)],
                        )
                    else:
                        nc.sync.dma_start(
                            xa, xT_r[:, kq * 8 : kq * 8 + 4, ts(tg, 512)]
                        )
                    xb = xpool.tile([P, 4, 512], BF16, tag="xt", name=f"xb{kq}{tg}")
                    nc.sync.dma_start(
                        xb, xT_r[:, kq * 8 + 4 : kq * 8 + 8, ts(tg, 512)]
                    )
                    return xa, xb

                # first x tiles go on the sync queue ahead of the wv
                # sync-tail sixteenths
                x00 = load_x(0, 0, first=True)
                wv_sb = load_w(wvT, sync_from=12)
                wq_sb = load_w(wqT)

                # --- v pass (x stationary) ---
                for tg in range(NTG):
                    vbanks = [
                        psacc.tile([P, DL], F32, tag="acc", name=f"vac{tg}{u}")
                        for u in range(4)
                    ]
                    for kq in range(4):
                        xa, xb = x00 if (tg == 0 and kq == 0) else load_x(kq, tg)
                        for u in range(4):
                            for kc in range(8):
                                xt = xa if kc < 4 else xb
                                nc.tensor.matmul(
                                    vbanks[u],
                                    xt[:, kc % 4, ts(u, P)],
                                    wv_sb[kq * 4 + kc // 2][:, kc % 2, :],
                                    start=(kq == 0 and kc == 0),
                                    stop=(kq == 3 and kc == 7),
                                )
                    for u in range(4):
                        nc.scalar.copy(vres[:, tg * 4 + u, :], vbanks[u])

                # --- q/k passes (weight stationary + rope) ---
                # rope tail (pair-swap matmul + mixes) deferred one tg so the
                # swap matmuls never make the PE wait on the ACT copies.
                def rope_finish(qraws, dst, tg):
                    stg = tg % (S // 512)  # rope position repeats per batch
                    tmps = []
                    for d in range(HL):
                        qsw = pssw.tile([P, 512], F32, tag="sw", name=f"sw{tg}{d}")
                        nc.tensor.matmul(
                            qsw, psw_sb, qraws[d], start=True, stop=True
                        )
                        tmp = rp.tile([P, 512], BF16, tag="rtmp", bufs=5,
                                      name=f"rt{tg}{d}")
                        nc.vector.tensor_tensor(
                            tmp, qsw, sin_sb[:, ts(stg, 512)], ALU.mult
                        )
                        tmps.append(tmp)
                    for d in range(HL):
                        dslc = dst[:, d, ts(tg, 512)]
                        nc.vector.tensor_tensor(
                            dslc, qraws[d], cos_sb[:, ts(stg, 512)], ALU.mult
                        )
                        nc.vector.tensor_tensor(dslc, dslc, tmps[d], ALU.add)

                wk_sb = load_w(wkT)
                pend_rope = None
                for proj in range(2):
                    w_sb = wq_sb if proj == 0 else wk_sb
                    dst = qres if proj == 0 else kres
                    for tg in range(NTG):
                        banks = [
                            psacc.tile([P, 512], F32, tag="acc", name=f"ac{proj}{tg}{d}")
                            for d in range(HL)
                        ]
                        for kq in range(4):
                            xa, xb = load_x(kq, tg)
                            for d in range(HL):
                                for kc in range(8):
                                    xt = xa if kc < 4 else xb
                                    nc.tensor.matmul(
                                        banks[d],
                                        w_sb[kq * 4 + kc // 2][:, kc % 2, ts(d, P)],
                                        xt[:, kc % 4, :],
                                        start=(kq == 0 and kc == 0),
                                        stop=(kq == 3 and kc == 7),
                                    )
                        if pend_rope is not None:
                            pend_rope()
                        qraws = []
                        for d in range(HL):
                            # PSUM -> SBUF bf16 raw copy (frees the acc bank)
                            qraw = rp.tile([P, 512], BF16, tag="qraw", bufs=6,
                                           name=f"qr{proj}{tg}{d}")
                            nc.scalar.copy(qraw, banks[d])
                            qraws.append(qraw)
                        pend_rope = (
                            lambda qraws=qraws, dst=dst, tg=tg: rope_finish(
                                qraws, dst, tg
                            )
                        )
                if pend_rope is not None:
                    pend_rope()

            # ============ Phase 2+3 shared residents ============
            with (
                tc.tile_pool(name="c2", bufs=1) as c2,
                tc.tile_pool(name="wop", bufs=1) as wop,
            ):
                ctx_sbuf = c2.tile([P, HL, T], BF16)
                # wo fully resident; DMA rides out during phase 2
                wo_sb = wop.tile([P, HL, D], BF16, tag="wo3")
                nc.scalar.dma_start(wo_sb, woT_r)

                # ============ Phase 2: attention ============
                with (
                    tc.tile_pool(name="mpool", bufs=1) as mpool,
                    tc.tile_pool(name="ppool", bufs=8) as ppool,
                    tc.tile_pool(name="apool", bufs=2) as apool,
                    tc.tile_pool(name="pssc", bufs=4, space="PSUM") as pssc,
                    tc.tile_pool(name="psctx", bufs=2, space="PSUM") as psctx,
                    tc.tile_pool(name="psden", bufs=2, space="PSUM") as psden,
                ):
                    tri_sb = mpool.tile([P, P], BF16)
                    nc.gpsimd.dma_start(tri_sb, trid.ap())

                    state = {"pend_tail": None}

                    def tail(den, ctx_ps, h, tcol):
                        """Per-(b,qt,h) context normalization."""
                        rec = apool.tile([P, 512], F32, tag="rec")
                        scr = apool.tile([P, 512], F32, tag="rscr")
                        nc.vector.reciprocal_approx_accurate(rec, den, scr)
                        nc.vector.tensor_tensor(
                            ctx_sbuf[:, h, ts(tcol, 512)], ctx_ps, rec, ALU.mult
                        )

                    for b in range(B):
                        for qt in range(NQT):
                            # masked (windowed) blocks first, ascending q0,
                            # then full blocks
                            blks = []
                            for kc in range(NKC):
                                if blocks[qt][kc] is not None:
                                    q0, tri = blocks[qt][kc]
                                    assert tri or q0 == 0
                                    blks.append((q0, tri, kc))
                            blks.sort(key=lambda x: (not x[1], x[0]))
                            assert blks[0][0] == 0, "first block must cover q0=0"
                            for h in range(HL):
                                qoff = (b * NQT + qt) * 512
                                ctx_ps = psctx.tile([P, 512], F32, tag="ctx")
                                den = psden.tile([P, 512], F32, tag="den")
                                inflight = []

                                def finish(i):
                                    q0, tri, kc, sps = inflight[i]
                                    w = 512 - q0
                                    pT = ppool.tile([P, 512], BF16, tag="pT")
                                    nc.scalar.activation(
                                        pT[:, :w], sps[:, :w], ACTF.Exp,
                                        scale=inv_sqrt_d,
                                    )
                                    if tri:
                                        # zero the above-diagonal triangle of
                                        # the leading 128 cols (Pool engine)
                                        nc.gpsimd.tensor_tensor(
                                            pT[:, 0:P], pT[:, 0:P], tri_sb,
                                            ALU.mult,
                                        )
                                    first = i == 0
                                    last = i == len(blks) - 1
                                    nc.tensor.matmul(
                                        ctx_ps[:, q0:],
                                        vres[:, b * NKC + kc, ts(h, P)],
                                        pT[:, :w],
                                        start=first,
                                        stop=last,
                                    )
                                    nc.tensor.matmul(
                                        den[:, q0:], ones_mat, pT[:, :w],
                                        start=first, stop=last,
                                    )
                                    if state["pend_tail"] is not None:
                                        state["pend_tail"]()
                                        state["pend_tail"] = None

                                nfin = 0
                                for bi, (q0, tri, kc) in enumerate(blks):
                                    sps = pssc.tile([P, 512], F32, tag="sc")
                                    w = 512 - q0
                                    nc.tensor.matmul(
                                        sps[:, :w],
                                        kres[:, h, b * S + kc * P : b * S + (kc + 1) * P],
                                        qres[:, h, qoff + q0 : qoff + 512],
                                        start=True,
                                        stop=True,
                                    )
                                    inflight.append((q0, tri, kc, sps))
                                    if len(inflight) - nfin >= 4:
                                        finish(nfin)
                                        nfin += 1
                                while nfin < len(inflight):
                                    finish(nfin)
                                    nfin += 1
                                state["pend_tail"] = (
                                    lambda den=den, ctx_ps=ctx_ps, h=h,
                                    tcol=b * NQT + qt: tail(den, ctx_ps, h, tcol)
                                )
                    state["pend_tail"]()

                # ===== Phase 3: output projection (wo stationary) =====
                with (
                    tc.tile_pool(name="ost", bufs=4) as ost,
                    tc.tile_pool(name="pso", bufs=4, space="PSUM") as pso,
                ):
                    dmaq = [nc.sync, nc.gpsimd, nc.scalar]
                    for tg in range(NTG):
                        for ep in range(D // P // 2):
                            ps_o = pso.tile([P, 2, 512], F32, tag="po")
                            for half in range(2):
                                es = ep * 2 + half
                                for dl in range(HL):
                                    nc.tensor.matmul(
                                        ps_o[:, half, :],
                                        wo_sb[:, dl, ts(es, P)],
                                        ctx_sbuf[:, dl, ts(tg, 512)],
                                        start=(dl == 0),
                                        stop=(dl == HL - 1),
                                    )
                            st = ost.tile([P, 2, 512], BF16, tag="ost")
                            i = tg * 16 + ep
                            # split PSUM->SBUF copies across ACT and DVE
                            if i % 2 == 0:
                                nc.scalar.copy(st, ps_o)
                            else:
                                nc.vector.tensor_copy(st, ps_o)
                            dmaq[i % 3].dma_start(
                                outT_r[:, ep * 2 : ep * 2 + 2, ts(tg, 512)], st
                            )

    nc.finalize()
    return nc


def kernel(x, wq, wk, wv, wo, cos, sin, mask):
    B, S, D = x.shape
    H = D // P
    NCORES = 8
    HL = H // NCORES
    DL = HL * P
    T = B * S

    import ml_dtypes

    BF = ml_dtypes.bfloat16

    x = np.asarray(x, dtype=np.float32)
    xT = np.ascontiguousarray(x.reshape(T, D).T).astype(BF)
    cos = np.asarray(cos, dtype=np.float32)
    sin = np.asarray(sin, dtype=np.float32)

    # rope tables on [hd-partition, token-free] layout
    cosE = np.repeat(cos.T, 2, axis=0).astype(BF)          # [128, S]
    sinS = np.empty((P, S), dtype=np.float32)              # signed sin
    sinS[0::2] = -sin.T
    sinS[1::2] = sin.T
    sinS = sinS.astype(BF)
    pswap = np.zeros((P, P), dtype=np.float32)
    for r in range(P):
        pswap[r, r ^ 1] = 1.0
    pswap = pswap.astype(BF)
    tri = np.triu(np.ones((P, P), dtype=np.float32)).astype(BF)

    maskT = np.ascontiguousarray(np.asarray(mask, dtype=np.float32)[0, 0].T)
    blocks = _analyze_blocks(maskT, S, 512)

    nc = _build(B, S, D, HL, blocks)

    wq = np.asarray(wq, dtype=np.float32)
    wk = np.asarray(wk, dtype=np.float32)
    wv = np.asarray(wv, dtype=np.float32)
    wo = np.asarray(wo, dtype=np.float32)

    in_maps = []
    for c in range(NCORES):
        sl = slice(c * DL, (c + 1) * DL)
        m = {
            "xT": xT,
            "cosE": cosE,
            "sinS": sinS,
            "pswap": pswap,
            "tri": tri,
            "wqT": np.ascontiguousarray(wq[sl, :].T).astype(BF),
            "wkT": np.ascontiguousarray(wk[sl, :].T).astype(BF),
            "wvT": np.ascontiguousarray(wv[sl, :].T).astype(BF),
            "woT": np.ascontiguousarray(wo[:, sl].T).astype(BF),
        }
        in_maps.append(m)

    kwargs = {}
    if TRACE:
        kwargs = {"trace": True}
        if TRACE_DIR:
            kwargs["tmpdir"] = TRACE_DIR
    res = run_bass_kernel_spmd(nc, in_maps, core_ids=list(range(NCORES)), **kwargs)
    LAST_RESULT[0] = res

    acc = res.results[0]["outT"].astype(np.float32)
    for c in range(1, NCORES):
        acc += res.results[c]["outT"].astype(np.float32)
    return np.ascontiguousarray(acc.T).reshape(B, S, D)


# revision 41
# speedup vs baseline: 1.0123x; 1.0123x over previous
"""Trainium2 Bass kernel for nn_AbstractAttention (dense transformer attention
with RoPE, B=2 S=2048 D=4096 H=32), tensor-parallel over heads on 8 cores.

Single-pass bf16 design; all matmuls in "flipped" orientations (no PE
transposes):

  q^T/k^T [dl, t]  = (wT chunk)^T @ x^T      (weight stationary)
  v [t, e]         = (x^T chunk)^T @ wvT     (x stationary)
  s^T [k, q]       = (k^T chunk)^T @ q^T     (k stationary)
  ctx^T [hd, q]    = (v chunk)^T @ p^T       (v stationary)  <- accumulates
  out^T [e, t]     = (woT chunk)^T @ ctx^T   (wo stationary)

v3 changes over the 1.27ms baseline (-> ~1.12ms, 92% PE MFU):
  - causal trimming: score/exp/PV/den matmuls only stream each key-chunk's
    valid query window (exact; saves 15% of attention flops/exp columns).
  - the causal triangle is applied multiplicatively (0/1 bf16 tile) to the
    exp output on the GpSimd/Pool engine, so the DVE only runs softmax
    tails and the ACT->PV chain has no DVE hop.
  - ctx stays resident in SBUF (no DRAM round-trip before out-proj).
  - attention loops (b, qt, h) so ctx for token-group tg completes in tg
    order; block-level software pipelining with 4 score banks in flight
    and per-iteration tails deferred into the next iteration.
  - rope tails (pair-swap matmul + mixes) deferred one token-group so the
    PE never waits on the ACT copies.
  - phase 3: two-bank PSUM pairs, copies alternate ACT/DVE, out DMAs
    round-robin across 3 queues.
  - weights DMA'd in sixteenths, x tiles in halves (faster first matmul).

Measured per-phase (span/idle us): v 236/8, qk 468/6, attention 194/5,
out-proj 204/1. Attention is exp-throughput-bound on ACT; projections are
at the PE bf16 streaming limit, so the remaining headroom is ~2%.
(Tried and rejected: fp8 DoubleRow (e4m3 noise blows the 2e-2 gate),
Pool-engine denominator accumulation (tensor ops ~1.1us/tile, too slow),
paired 1024-wide exps (needs 6 score banks; PSUM only has 8 total and
ctx/den double-buffering needs 4), DVE denominator adds (attention is
ACT-bound, so removing PE work gains nothing and perturbs the pipeline).)

Softmax skips max-subtraction (scores are O(1)); the denominator is a
ones-row matmul partition-reduce riding the PE; normalization happens on
the small context via reciprocal on DVE.

Host: shards weights by head, preps bf16 transposed layouts, sums the 8
partial out^T tensors in fp32 and transposes back.
"""

import math

import numpy as np

import concourse.bacc as bacc
import concourse.mybir as mybir
from concourse.bass_utils import run_bass_kernel_spmd
from concourse.tile import TileContext

P = 128
F32 = mybir.dt.float32
BF16 = mybir.dt.bfloat16
ALU = mybir.AluOpType
ACTF = mybir.ActivationFunctionType

# set by test.py for profiling; grading path leaves these alone
TRACE = False
TRACE_DIR = None
LAST_RESULT = [None]


def _analyze_blocks(maskT, S, QT):
    """maskT: [S, S] (k, q). Per (qt, kc) block: None if fully masked, else
    (q0, tri) with q0 the first valid query column (suffix window) and tri
    True if the leading [128, 128] of the window needs the causal triangle.
    Requires a pure 0/-inf mask whose masked region per block is exactly a
    suffix-window + leading lower-triangle (true for causal)."""
    nqt, nkc = S // QT, S // P
    blocks = [[None] * nkc for _ in range(nqt)]
    tri = np.triu(np.ones((P, P), dtype=np.float32))
    neg = np.isneginf(maskT) | (maskT < -1e30)
    assert np.all((maskT == 0.0) | neg), "mask must be 0/-inf"
    for qt in range(nqt):
        for kc in range(nkc):
            sub = neg[kc * P : (kc + 1) * P, qt * QT : (qt + 1) * QT]
            col_masked = sub.all(axis=0)
            if col_masked.all():
                continue
            q0 = int(np.argmax(~col_masked))
            assert not col_masked[q0:].any() or col_masked[q0:].sum() == 0
            win = sub[:, q0:]
            if not win.any():
                blocks[qt][kc] = (q0, False)
            else:
                assert win.shape[1] >= P, "triangle wider than window"
                got = (~win[:, :P]).astype(np.float32)
                assert np.array_equal(got, tri), "non-causal block pattern"
                assert not win[:, P:].any()
                blocks[qt][kc] = (q0, True)
    return blocks


def _build(B, S, D, HL, blocks):
    """Per-core Bass program. HL local heads, DL=HL*128 local dims."""
    DL = HL * P
    T = B * S
    KD = D // P        # 32 K-chunks of the model dim
    NTG = T // 512     # 8 token groups of 512
    NKC = S // P       # 16 key chunks per batch
    NQT = S // 512     # 4 query tiles per batch
    inv_sqrt_d = 1.0 / math.sqrt(P)

    nc = bacc.Bacc(None, target_bir_lowering=False)

    xT = nc.declare_dram_parameter("xT", [D, T], BF16, isOutput=False)
    wqT = nc.declare_dram_parameter("wqT", [D, DL], BF16, isOutput=False)
    wkT = nc.declare_dram_parameter("wkT", [D, DL], BF16, isOutput=False)
    wvT = nc.declare_dram_parameter("wvT", [D, DL], BF16, isOutput=False)
    woT = nc.declare_dram_parameter("woT", [DL, D], BF16, isOutput=False)
    cosE = nc.declare_dram_parameter("cosE", [P, S], BF16, isOutput=False)
    sinS = nc.declare_dram_parameter("sinS", [P, S], BF16, isOutput=False)
    pswap = nc.declare_dram_parameter("pswap", [P, P], BF16, isOutput=False)
    trid = nc.declare_dram_parameter("tri", [P, P], BF16, isOutput=False)
    outT = nc.declare_dram_parameter("outT", [D, T], BF16, isOutput=True)

    ts = lambda i, s: slice(i * s, (i + 1) * s)

    xT_r = xT.ap().rearrange("(o p) t -> p o t", p=P)
    woT_r = woT.ap().rearrange("(o p) e -> p o e", p=P)
    outT_r = outT.ap().rearrange("(o p) t -> p o t", p=P)

    with TileContext(nc) as tc:
        with (
            tc.tile_pool(name="res", bufs=1) as res,
            tc.tile_pool(name="consts", bufs=1) as consts,
        ):
            # resident q^T/k^T (roped, bf16) and v
            qres = res.tile([P, HL, T], BF16)
            kres = res.tile([P, HL, T], BF16)
            vres = res.tile([P, T // P, DL], BF16)  # [k-in-chunk, t-chunk, (h,hd)]

            ones_mat = consts.tile([P, P], BF16)
            nc.vector.memset(ones_mat, 1.0)

            # ============ Phase 1: projections, order v -> q -> k ============
            with (
                tc.tile_pool(name="p1c", bufs=1) as p1c,
                tc.tile_pool(name="wpool", bufs=2) as wpool,
                tc.tile_pool(name="xpool", bufs=7) as xpool,
                tc.tile_pool(name="rp", bufs=1) as rp,
                tc.tile_pool(name="psacc", bufs=6, space="PSUM") as psacc,
                tc.tile_pool(name="pssw", bufs=2, space="PSUM") as pssw,
            ):
                cos_sb = p1c.tile([P, S], BF16)
                nc.gpsimd.dma_start(cos_sb, cosE.ap())
                sin_sb = p1c.tile([P, S], BF16)
                nc.gpsimd.dma_start(sin_sb, sinS.ap())
                psw_sb = p1c.tile([P, P], BF16)
                nc.gpsimd.dma_start(psw_sb, pswap.ap())

                def load_w(src):
                    # scalar-queue issue: keeps x tiles (sync queue) unblocked
                    # sixteenth-granularity so the first matmuls start early
                    six = []
                    r = src.ap().rearrange("(o p) n -> p o n", p=P)
                    for ke in range(16):
                        t = wpool.tile([P, 2, DL], BF16, tag=f"w{ke}")
                        if ke == 0:
                            # split the very first tile so matmul 0 can
                            # start after a quarter-MB
                            nc.scalar.dma_start(t[:, 0, :], r[:, ke * 2, :])
                            nc.scalar.dma_start(t[:, 1, :], r[:, ke * 2 + 1, :])
                        else:
                            nc.scalar.dma_start(t, r[:, ts(ke, 2), :])
                        six.append(t)
                    return six

                wv_sb = load_w(wvT)
                wq_sb = load_w(wqT)

                def load_x(kq, tg, first=False):
                    # two half tiles so the first matmuls start earlier
                    xa = xpool.tile([P, 4, 512], BF16, tag="xt", name=f"xa{kq}{tg}")
                    if first:
                        # quarter-granularity for the very first load so
                        # matmul 0 starts as early as possible
                        nc.sync.dma_start(
                            xa[:, 0:2, :], xT_r[:, kq * 8 : kq * 8 + 2, ts(tg, 512)]
                        )
                        nc.sync.dma_start(
                            xa[:, 2:4, :],
                            xT_r[:, kq * 8 + 2 : kq * 8 + 4, ts(tg, 512)],
                        )
                    else:
                        nc.sync.dma_start(
                            xa, xT_r[:, kq * 8 : kq * 8 + 4, ts(tg, 512)]
                        )
                    xb = xpool.tile([P, 4, 512], BF16, tag="xt", name=f"xb{kq}{tg}")
                    nc.sync.dma_start(
                        xb, xT_r[:, kq * 8 + 4 : kq * 8 + 8, ts(tg, 512)]
                    )
                    return xa, xb

                # --- v pass (x stationary) ---
                for tg in range(NTG):
                    vbanks = [
                        psacc.tile([P, DL], F32, tag="acc", name=f"vac{tg}{u}")
                        for u in range(4)
                    ]
                    for kq in range(4):
                        xa, xb = load_x(kq, tg, first=(kq == 0 and tg == 0))
                        for u in range(4):
                            for kc in range(8):
                                xt = xa if kc < 4 else xb
                                nc.tensor.matmul(
                                    vbanks[u],
                                    xt[:, kc % 4, ts(u, P)],
                                    wv_sb[kq * 4 + kc // 2][:, kc % 2, :],
                                    start=(kq == 0 and kc == 0),
                                    stop=(kq == 3 and kc == 7),
                                )
                    for u in range(4):
                        nc.scalar.copy(vres[:, tg * 4 + u, :], vbanks[u])

                # --- q/k passes (weight stationary + rope) ---
                # rope tail (pair-swap matmul + mixes) deferred one tg so the
                # swap matmuls never make the PE wait on the ACT copies.
                def rope_finish(qraws, dst, tg):
                    stg = tg % (S // 512)  # rope position repeats per batch
                    tmps = []
                    for d in range(HL):
                        qsw = pssw.tile([P, 512], F32, tag="sw", name=f"sw{tg}{d}")
                        nc.tensor.matmul(
                            qsw, psw_sb, qraws[d], start=True, stop=True
                        )
                        tmp = rp.tile([P, 512], BF16, tag="rtmp", bufs=5,
                                      name=f"rt{tg}{d}")
                        nc.vector.tensor_tensor(
                            tmp, qsw, sin_sb[:, ts(stg, 512)], ALU.mult
                        )
                        tmps.append(tmp)
                    for d in range(HL):
                        dslc = dst[:, d, ts(tg, 512)]
                        nc.vector.tensor_tensor(
                            dslc, qraws[d], cos_sb[:, ts(stg, 512)], ALU.mult
                        )
                        nc.vector.tensor_tensor(dslc, dslc, tmps[d], ALU.add)

                wk_sb = load_w(wkT)
                pend_rope = None
                for proj in range(2):
                    w_sb = wq_sb if proj == 0 else wk_sb
                    dst = qres if proj == 0 else kres
                    for tg in range(NTG):
                        banks = [
                            psacc.tile([P, 512], F32, tag="acc", name=f"ac{proj}{tg}{d}")
                            for d in range(HL)
                        ]
                        for kq in range(4):
                            xa, xb = load_x(kq, tg)
                            for d in range(HL):
                                for kc in range(8):
                                    xt = xa if kc < 4 else xb
                                    nc.tensor.matmul(
                                        banks[d],
                                        w_sb[kq * 4 + kc // 2][:, kc % 2, ts(d, P)],
                                        xt[:, kc % 4, :],
                                        start=(kq == 0 and kc == 0),
                                        stop=(kq == 3 and kc == 7),
                                    )
                        if pend_rope is not None:
                            pend_rope()
                        qraws = []
                        for d in range(HL):
                            # PSUM -> SBUF bf16 raw copy (frees the acc bank)
                            qraw = rp.tile([P, 512], BF16, tag="qraw", bufs=6,
                                           name=f"qr{proj}{tg}{d}")
                            nc.scalar.copy(qraw, banks[d])
                            qraws.append(qraw)
                        pend_rope = (
                            lambda qraws=qraws, dst=dst, tg=tg: rope_finish(
                                qraws, dst, tg
                            )
                        )
                if pend_rope is not None:
                    pend_rope()

            # ============ Phase 2+3 shared residents ============
            with (
                tc.tile_pool(name="c2", bufs=1) as c2,
                tc.tile_pool(name="wop", bufs=1) as wop,
            ):
                ctx_sbuf = c2.tile([P, HL, T], BF16)
                # wo fully resident; DMA rides out during phase 2
                wo_sb = wop.tile([P, HL, D], BF16, tag="wo3")
                nc.scalar.dma_start(wo_sb, woT_r)

                # ============ Phase 2: attention ============
                with (
                    tc.tile_pool(name="mpool", bufs=1) as mpool,
                    tc.tile_pool(name="ppool", bufs=8) as ppool,
                    tc.tile_pool(name="apool", bufs=2) as apool,
                    tc.tile_pool(name="pssc", bufs=4, space="PSUM") as pssc,
                    tc.tile_pool(name="psctx", bufs=2, space="PSUM") as psctx,
                    tc.tile_pool(name="psden", bufs=2, space="PSUM") as psden,
                ):
                    tri_sb = mpool.tile([P, P], BF16)
                    nc.gpsimd.dma_start(tri_sb, trid.ap())

                    state = {"pend_tail": None}

                    def tail(den, ctx_ps, h, tcol):
                        """Per-(b,qt,h) context normalization."""
                        rec = apool.tile([P, 512], F32, tag="rec")
                        scr = apool.tile([P, 512], F32, tag="rscr")
                        nc.vector.reciprocal_approx_accurate(rec, den, scr)
                        nc.vector.tensor_tensor(
                            ctx_sbuf[:, h, ts(tcol, 512)], ctx_ps, rec, ALU.mult
                        )

                    for b in range(B):
                        for qt in range(NQT):
                            # masked (windowed) blocks first, ascending q0,
                            # then full blocks
                            blks = []
                            for kc in range(NKC):
                                if blocks[qt][kc] is not None:
                                    q0, tri = blocks[qt][kc]
                                    assert tri or q0 == 0
                                    blks.append((q0, tri, kc))
                            blks.sort(key=lambda x: (not x[1], x[0]))
                            assert blks[0][0] == 0, "first block must cover q0=0"
                            for h in range(HL):
                                qoff = (b * NQT + qt) * 512
                                ctx_ps = psctx.tile([P, 512], F32, tag="ctx")
                                den = psden.tile([P, 512], F32, tag="den")
                                inflight = []

                                def finish(i):
                                    q0, tri, kc, sps = inflight[i]
                                    w = 512 - q0
                                    pT = ppool.tile([P, 512], BF16, tag="pT")
                                    nc.scalar.activation(
                                        pT[:, :w], sps[:, :w], ACTF.Exp,
                                        scale=inv_sqrt_d,
                                    )
                                    if tri:
                                        # zero the above-diagonal triangle of
                                        # the leading 128 cols (Pool engine)
                                        nc.gpsimd.tensor_tensor(
                                            pT[:, 0:P], pT[:, 0:P], tri_sb,
                                            ALU.mult,
                                        )
                                    first = i == 0
                                    last = i == len(blks) - 1
                                    nc.tensor.matmul(
                                        ctx_ps[:, q0:],
                                        vres[:, b * NKC + kc, ts(h, P)],
                                        pT[:, :w],
                                        start=first,
                                        stop=last,
                                    )
                                    nc.tensor.matmul(
                                        den[:, q0:], ones_mat, pT[:, :w],
                                        start=first, stop=last,
                                    )
                                    if state["pend_tail"] is not None:
                                        state["pend_tail"]()
                                        state["pend_tail"] = None

                                nfin = 0
                                for bi, (q0, tri, kc) in enumerate(blks):
                                    sps = pssc.tile([P, 512], F32, tag="sc")
                                    w = 512 - q0
                                    nc.tensor.matmul(
                                        sps[:, :w],
                                        kres[:, h, b * S + kc * P : b * S + (kc + 1) * P],
                                        qres[:, h, qoff + q0 : qoff + 512],
                                        start=True,
                                        stop=True,
                                    )
                                    inflight.append((q0, tri, kc, sps))
                                    if len(inflight) - nfin >= 4:
                                        finish(nfin)
                                        nfin += 1
                                while nfin < len(inflight):
                                    finish(nfin)
                                    nfin += 1
                                state["pend_tail"] = (
                                    lambda den=den, ctx_ps=ctx_ps, h=h,
                                    tcol=b * NQT + qt: tail(den, ctx_ps, h, tcol)
                                )
                    state["pend_tail"]()

                # ===== Phase 3: output projection (wo stationary) =====
                with (
                    tc.tile_pool(name="ost", bufs=4) as ost,
                    tc.tile_pool(name="pso", bufs=4, space="PSUM") as pso,
                ):
                    dmaq = [nc.sync, nc.gpsimd, nc.scalar]
                    for tg in range(NTG):
                        for ep in range(D // P // 2):
                            ps_o = pso.tile([P, 2, 512], F32, tag="po")
                            for half in range(2):
                                es = ep * 2 + half
                                for dl in range(HL):
                                    nc.tensor.matmul(
                                        ps_o[:, half, :],
                                        wo_sb[:, dl, ts(es, P)],
                                        ctx_sbuf[:, dl, ts(tg, 512)],
                                        start=(dl == 0),
                                        stop=(dl == HL - 1),
                                    )
                            st = ost.tile([P, 2, 512], BF16, tag="ost")
                            i = tg * 16 + ep
                            # split PSUM->SBUF copies across ACT and DVE
                            if i % 2 == 0:
                                nc.scalar.copy(st, ps_o)
                            else:
                                nc.vector.tensor_copy(st, ps_o)
                            dmaq[i % 3].dma_start(
                                outT_r[:, ep * 2 : ep * 2 + 2, ts(tg, 512)], st
                            )

    nc.finalize()
    return nc


def kernel(x, wq, wk, wv, wo, cos, sin, mask):
    B, S, D = x.shape
    H = D // P
    NCORES = 8
    HL = H // NCORES
    DL = HL * P
    T = B * S

    import ml_dtypes

    BF = ml_dtypes.bfloat16

    x = np.asarray(x, dtype=np.float32)
    xT = np.ascontiguousarray(x.reshape(T, D).T).astype(BF)
    cos = np.asarray(cos, dtype=np.float32)
    sin = np.asarray(sin, dtype=np.float32)

    # rope tables on [hd-partition, token-free] layout
    cosE = np.repeat(cos.T, 2, axis=0).astype(BF)          # [128, S]
    sinS = np.empty((P, S), dtype=np.float32)              # signed sin
    sinS[0::2] = -sin.T
    sinS[1::2] = sin.T
    sinS = sinS.astype(BF)
    pswap = np.zeros((P, P), dtype=np.float32)
    for r in range(P):
        pswap[r, r ^ 1] = 1.0
    pswap = pswap.astype(BF)
    tri = np.triu(np.ones((P, P), dtype=np.float32)).astype(BF)

    maskT = np.ascontiguousarray(np.asarray(mask, dtype=np.float32)[0, 0].T)
    blocks = _analyze_blocks(maskT, S, 512)

    nc = _build(B, S, D, HL, blocks)

    wq = np.asarray(wq, dtype=np.float32)
    wk = np.asarray(wk, dtype=np.float32)
    wv = np.asarray(wv, dtype=np.float32)
    wo = np.asarray(wo, dtype=np.float32)

    in_maps = []
    for c in range(NCORES):
        sl = slice(c * DL, (c + 1) * DL)
        m = {
            "xT": xT,
            "cosE": cosE,
            "sinS": sinS,
            "pswap": pswap,
            "tri": tri,
            "wqT": np.ascontiguousarray(wq[sl, :].T).astype(BF),
            "wkT": np.ascontiguousarray(wk[sl, :].T).astype(BF),
            "wvT": np.ascontiguousarray(wv[sl, :].T).astype(BF),
            "woT": np.ascontiguousarray(wo[:, sl].T).astype(BF),
        }
        in_maps.append(m)

    kwargs = {}
    if TRACE:
        kwargs = {"trace": True}
        if TRACE_DIR:
            kwargs["tmpdir"] = TRACE_DIR
    res = run_bass_kernel_spmd(nc, in_maps, core_ids=list(range(NCORES)), **kwargs)
    LAST_RESULT[0] = res

    acc = res.results[0]["outT"].astype(np.float32)
    for c in range(1, NCORES):
        acc += res.results[c]["outT"].astype(np.float32)
    return np.ascontiguousarray(acc.T).reshape(B, S, D)
